# revision 6
# baseline (speedup 1.0000x reference)
"""DropSphereNd Trainium2 kernel.

Full computation (per sample n, channels c):
    activ = embeds @ table                      # [n, c]
    t     = 17th-smallest(activ, axis=1)        # [n, 1]
    out   = x * (activ >= t) * c/(c-16)

Sharding: data-parallel over batch n across 8 cores (x/embeds sharded,
table replicated).  Per core: x shard [8, 256, 56, 56] viewed as
[2048, 3136]; the mask is computed on-device (tiny matmul + iterative
min-extraction) and applied as a per-partition scalar multiply while
streaming x through SBUF.

The kernel is DMA-fabric bound: the 16 SDMA engines must carry
25.7 MB in + 25.7 MB out.  Loads ride the SP HWDGE ring, stores the
ACT ring; engines round-robin between them, so throughput depends on
both streams staying dense.  The mask critical path decides when the
store stream opens: embeds is loaded contiguously and transposed
on-chip by the PE (a DMA-transposed [16,8] load is 128 sub-512B
descriptors that take ~17 us against bulk traffic), and the first
sample is loaded/multiplied/stored in four quarter-tiles so stores
flow as soon as the mask lands (~12 us instead of ~34 us).  The last
sample's mask-multiply + store run in two half-tiles to shrink the
end-of-pipeline serialization.  Measured: ~98-117 us vs ~113-155 us
for the coarse-tiled baseline (variance is HBM/fabric contention).

Raw bass (no Tile): all cross-engine deps use standalone wait_ge
sequencer commands; DMA sems tick in units of 16 (one per SDMA engine).

Engine plan:
  SP  (nc.sync)   - x tile loads
  ACT (nc.scalar) - small input DMAs (embeds, ident, table) + tile stores
  PE  (nc.tensor) - embeds transpose + projection matmul + 2 mask-transpose matmuls
  DVE (nc.vector) - threshold search, mask build, streaming multiplies
"""

import sys

if "/opt/trn_rl_repo" not in sys.path:
    sys.path.insert(0, "/opt/trn_rl_repo")

from contextlib import ExitStack

import numpy as np

import concourse.bass as bass
from concourse import mybir
from concourse.bass_utils import run_bass_kernel_spmd

N, C, H, W = 64, 256, 56, 56
HW = H * W  # 3136
E = 16
NCORES = 8
NLOC = N // NCORES  # 8 samples per core
INDEX = 16  # ceil(C ** 0.5)
SCALE = float(C) / (C - INDEX)
F32 = mybir.dt.float32
BUFS = 7  # x-tile ring slots (25 KB/partition each)

# mask-multiply + store units (sample, f0, f1): the first sample is
# processed in four quarter-tiles so the store stream opens as soon as
# the mask is ready; the last sample in two half-tiles so the final
# mul+store tail is short.
Q4 = HW // 4
UNITS = (
    [(0, q * Q4, (q + 1) * Q4) for q in range(4)]
    + [(s, 0, HW) for s in range(1, 7)]
    + [(7, 0, HW // 2), (7, HW // 2, HW)]
)
NU = len(UNITS)  # 12
# load units: sample 0 in quarters (so mul 0 unblocks early), rest whole
LOADS = [(0, q * Q4, (q + 1) * Q4) for q in range(4)] + [
    (s, 0, HW) for s in range(1, 8)
]
NL = len(LOADS)  # 11

_NC_CACHE = {}


def _build_nc() -> bass.Bass:
    # detect_race_conditions only affects the interpreter: its raw-bass model
    # has no same-engine program-order edges, so every chained DVE op would be
    # flagged.  Cross-engine ordering is handled by the explicit sems below.
    nc = bass.Bass(detect_race_conditions=False)
    x = nc.dram_tensor("x", [NLOC * C, HW], F32, kind="ExternalInput")
    emb = nc.dram_tensor("embeds", [NLOC, E], F32, kind="ExternalInput")
    tab = nc.dram_tensor("table", [E, C], F32, kind="ExternalInput")
    out = nc.dram_tensor("out", [NLOC * C, HW], F32, kind="ExternalOutput")
    ident_d = nc.inline_tensor(np.eye(NLOC, dtype=np.float32), name="ident8")

    # row r = t*256 + 2*p + parity  ->  sample t, channel c = 2*p + parity
    x_t = x[:, :].rearrange("(t p two) f -> t p two f", p=128, two=2)
    o_t = out[:, :].rearrange("(t p two) f -> t p two f", p=128, two=2)

    with ExitStack() as ctx:
        sb = lambda name, shape: ctx.enter_context(nc.sbuf_tensor(name, shape, F32))
        ps = lambda name, shape: ctx.enter_context(nc.psum_tensor(name, shape, F32))

        tab_s = sb("tab_s", [E, C])
        embS = sb("embS", [NLOC, E])  # embeds as loaded, [n, e]
        embT = sb("embT", [E, NLOC])  # embeds transposed on-chip, [e, n]
        ident = sb("ident", [NLOC, NLOC])
        v = sb("v", [NLOC, C])
        v2 = sb("v2", [NLOC, C])
        mx = sb("mx", [NLOC, 8])
        m_even = sb("m_even", [NLOC, C // 2])
        m_odd = sb("m_odd", [NLOC, C // 2])
        mA = sb("mA", [C // 2, NLOC])
        mB = sb("mB", [C // 2, NLOC])
        xbuf = [sb(f"xbuf{i}", [128, 2, HW]) for i in range(BUFS)]

        activ_p = ps("activ_p", [NLOC, C])
        embT_p = ps("embT_p", [E, NLOC])
        mA_p = ps("mA_p", [C // 2, NLOC])
        mB_p = ps("mB_p", [C // 2, NLOC])

        ld_e = ctx.enter_context(nc.semaphore("ld_e"))
        ld_i = ctx.enter_context(nc.semaphore("ld_i"))
        ld_t = ctx.enter_context(nc.semaphore("ld_t"))
        fz = ctx.enter_context(nc.semaphore("fz"))
        dv = ctx.enter_context(nc.semaphore("dv"))
        pe = ctx.enter_context(nc.semaphore("pe"))
        # per-sample load sems / per-unit store sems: each carries exactly
        # one DMA, so value 16 unambiguously means "all 16 SDMA engines done"
        xs = [ctx.enter_context(nc.semaphore(f"xs{l}")) for l in range(NL)]
        ss = [ctx.enter_context(nc.semaphore(f"ss{u}")) for u in range(NU)]

        block = ctx.enter_context(nc.Block())

        # x loads get the SP ring to themselves: the first x descriptor
        # generates immediately instead of queueing behind the smalls.
        # Smalls ride the ACT ring, idle until stores begin.
        @block.sync
        def _(sync):
            seen = set()
            for l, (s, f0, f1) in enumerate(LOADS):
                if s >= BUFS and s not in seen:
                    # slot free once every store unit of sample s-BUFS drained
                    for u, (us, _, _) in enumerate(UNITS):
                        if us == s - BUFS:
                            sync.wait_ge(ss[u], 16)
                seen.add(s)
                sync.dma_start(
                    out=xbuf[s % BUFS][:, :, f0:f1], in_=x_t[s][:, :, f0:f1]
                ).then_inc(xs[l], 16)

        @block.tensor
        def _(tensor):
            tensor.wait_ge(ld_e, 16)
            tensor.wait_ge(ld_i, 16)
            # embT = embS.T @ I : on-chip transpose of the tiny embeds block
            tensor.matmul(
                embT_p[:, :], embS[:, :], ident[:, :], start=True, stop=True
            ).then_inc(pe, 1)
            tensor.wait_ge(fz, 1)  # embT copied to SBUF by DVE
            tensor.wait_ge(ld_t, 16)
            tensor.matmul(
                activ_p[:, :], embT[:, :], tab_s[:, :], start=True, stop=True
            ).then_inc(pe, 1)
            tensor.wait_ge(dv, 2)  # m_even + m_odd built
            tensor.matmul(
                mA_p[:, :], m_even[:, :], ident[:, :], start=True, stop=True
            ).then_inc(pe, 1)
            tensor.matmul(
                mB_p[:, :], m_odd[:, :], ident[:, :], start=True, stop=True
            ).then_inc(pe, 1)

        # The 16 smallest of activ == the 16 largest of v = -activ.  DVE's
        # max (top-8 per partition) + match_replace (zap those 8) drop them
        # in two rounds; surviving lanes keep their value, zapped lanes hold
        # MINV, so the mask is one compare against an immediate.  No
        # data-dependent scalar operands anywhere: TensorScalarPtr fetches
        # its scalar at sequencer dispatch (ahead of the DVE pipe), so only
        # mA/mB -- real pointer operands of the streaming muls -- need a
        # sem fence.
        MINV = -1.0e30

        @block.vector
        def _(vector):
            vector.wait_ge(pe, 1)
            vector.tensor_copy(embT[:, :], embT_p[:, :]).then_inc(fz, 1)
            vector.wait_ge(pe, 2)
            vector.tensor_scalar_mul(v[:, :], activ_p[:, :], -1.0)
            # match_replace prefetches its 8-value table at dispatch, ahead
            # of the DVE pipe -- fence each max before consuming it
            vector.max(mx[:, :], v[:, :]).then_inc(fz, 1)
            vector.wait_ge(fz, 2)
            vector.match_replace(
                out=v2[:, :], in_to_replace=mx[:, :], in_values=v[:, :],
                imm_value=MINV,
            )
            vector.max(mx[:, :], v2[:, :]).then_inc(fz, 1)
            vector.wait_ge(fz, 3)
            vector.match_replace(
                out=v2[:, :], in_to_replace=mx[:, :], in_values=v2[:, :],
                imm_value=MINV,
            )
            # keep[c] <=> v2[c] != MINV ; mask = keep * SCALE, channel-parity
            # split (immediate compare: real values are > MINV/2)
            v_pair = v2[:, :].rearrange("n (j two) -> n j two", two=2)
            for parity, m8 in ((0, m_even), (1, m_odd)):
                vector.tensor_scalar(
                    out=m8[:, :],
                    in0=v_pair[:, :, parity],
                    scalar1=MINV / 2,
                    scalar2=SCALE,
                    op0=mybir.AluOpType.is_ge,
                    op1=mybir.AluOpType.mult,
                ).then_inc(dv, 1)
            vector.wait_ge(pe, 4)
            vector.tensor_copy(mA[:, :], mA_p[:, :])
            vector.tensor_copy(mB[:, :], mB_p[:, :]).then_inc(dv, 1)
            vector.wait_ge(dv, 3)  # mA/mB committed before mul ptr-fetches
            for u, (s, f0, f1) in enumerate(UNITS):
                # the load unit covering (s, f0:f1): s0 quarters map 1:1,
                # other samples are single whole-tile loads
                l = u if s == 0 else 3 + s
                vector.wait_ge(xs[l], 16)
                xb = xbuf[s % BUFS]
                vector.tensor_scalar_mul(
                    xb[:, 0, f0:f1], xb[:, 0, f0:f1], mA[:, s : s + 1]
                )
                vector.tensor_scalar_mul(
                    xb[:, 1, f0:f1], xb[:, 1, f0:f1], mB[:, s : s + 1]
                ).then_inc(dv, 1)

        DV_BASE = 3  # dv value once masks + mA/mB copies are done

        @block.scalar
        def _(scalar):
            scalar.dma_start(out=embS[:, :], in_=emb[:, :]).then_inc(ld_e, 16)
            scalar.dma_start(out=ident[:, :], in_=ident_d[:, :]).then_inc(ld_i, 16)
            scalar.dma_start(out=tab_s[:, :], in_=tab[:, :]).then_inc(ld_t, 16)
            for u, (s, f0, f1) in enumerate(UNITS):
                scalar.wait_ge(dv, DV_BASE + (u + 1))  # both muls of unit u done
                scalar.dma_start(
                    out=o_t[s][:, :, f0:f1], in_=xbuf[s % BUFS][:, :, f0:f1]
                ).then_inc(ss[u], 16)

    return nc


def _get_nc() -> bass.Bass:
    if "nc" not in _NC_CACHE:
        _NC_CACHE["nc"] = _build_nc()
    return _NC_CACHE["nc"]


def _in_maps(x, embeds, table):
    x = np.ascontiguousarray(np.asarray(x, dtype=np.float32))
    embeds = np.ascontiguousarray(np.asarray(embeds, dtype=np.float32))
    table = np.ascontiguousarray(np.asarray(table, dtype=np.float32))
    maps = []
    for i in range(NCORES):
        maps.append(
            {
                "x": x[i * NLOC : (i + 1) * NLOC].reshape(NLOC * C, HW),
                "embeds": embeds[i * NLOC : (i + 1) * NLOC],
                "table": table,
            }
        )
    return maps


def kernel(x, embeds, table):
    nc = _get_nc()
    res = run_bass_kernel_spmd(nc, _in_maps(x, embeds, table), list(range(NCORES)))
    shards = [
        np.asarray(res.results[i]["out"]).reshape(NLOC, C, H, W)
        for i in range(NCORES)
    ]
    return np.concatenate(shards, axis=0)


def kernel_profiled(x, embeds, table, **trace_kwargs):
    """Same as kernel() but with NTFF tracing; returns (output, BassKernelResults)."""
    nc = _get_nc()
    res = run_bass_kernel_spmd(
        nc, _in_maps(x, embeds, table), list(range(NCORES)), trace=True, **trace_kwargs
    )
    shards = [
        np.asarray(res.results[i]["out"]).reshape(NLOC, C, H, W)
        for i in range(NCORES)
    ]
    return np.concatenate(shards, axis=0), res
